# revision 14
# baseline (speedup 1.0000x reference)
"""AttnDecoderRNN single-step decoder on 8 TRN2 NeuronCores.

Sharding: hidden features (H=1024 -> 128/core) for emb/GRU/attention;
vocab rows (50257 -> 6656 padded/core) for W_out logits + log_softmax.
All cores run one SPMD graph; per-core differences come via input data.

Queues: sync = weight preloads + W_out stream only (no chain waits);
gpsimd = collectives + all chain/epilogue DMAs (serial anyway).
"""
import numpy as np
import ml_dtypes

import concourse.bass as bass
import concourse.bacc as bacc
import concourse.mybir as mybir
from concourse import tile
from concourse import bass_isa
from concourse.bass_utils import run_bass_kernel_spmd

F32 = mybir.dt.float32
BF16 = mybir.dt.bfloat16
BF16NP = ml_dtypes.bfloat16
F8 = mybir.dt.float8e4
F8NP = ml_dtypes.float8_e4m3
SCALE_W = 64.0
SCALE_A = 16.0
INV_SCALE = 1.0 / (SCALE_W * SCALE_A)
AF = mybir.ActivationFunctionType
X = mybir.AxisListType.X

NC_ = 8
V, H, S = 50257, 1024, 2048
HS = H // NC_            # 128 features per core
KX = 16                  # x0 chunks (2048/128)
KH = 8                   # h chunks (1024/128)
VT = 512                 # vocab tile free dim
NVT = 13                 # vocab tiles per core
VPC = VT * NVT           # 6656 padded vocab rows per core
VP = VPC * NC_           # 53248
LT = VPC // 128          # 52 logits per partition in [128, 52] layout


def build_graph(idx: int, w_bufs: int = 8, debug: bool = False):
    nc = bacc.Bacc("TRN2", debug=False, num_devices=NC_)
    rg = [list(range(NC_))]

    # ---- inputs (p-major layouts so every DMA row is contiguous) ----
    emb_f = nc.declare_dram_parameter("emb_f", [V, HS], F32, isOutput=False)
    ctx_in = nc.declare_dram_parameter("ctx_in", [128, KH], F32, isOutput=False)
    hp0 = nc.declare_dram_parameter("hp0", [128, KH], F32, isOutput=False)
    hp1 = nc.declare_dram_parameter("hp1", [128, KH], F32, isOutput=False)
    hp0f = nc.declare_dram_parameter("hp0f", [HS, 1], F32, isOutput=False)
    hp1f = nc.declare_dram_parameter("hp1f", [HS, 1], F32, isOutput=False)
    wih0 = nc.declare_dram_parameter("wih0", [128, KX, 384], BF16, isOutput=False)
    whh0 = nc.declare_dram_parameter("whh0", [128, KH, 384], BF16, isOutput=False)
    wih1 = nc.declare_dram_parameter("wih1", [128, KH, 384], BF16, isOutput=False)
    whh1 = nc.declare_dram_parameter("whh1", [128, KH, 384], BF16, isOutput=False)
    bih0 = nc.declare_dram_parameter("bih0", [128, 3], F32, isOutput=False)
    bhh0 = nc.declare_dram_parameter("bhh0", [128, 3], F32, isOutput=False)
    bih1 = nc.declare_dram_parameter("bih1", [128, 3], F32, isOutput=False)
    bhh1 = nc.declare_dram_parameter("bhh1", [128, 3], F32, isOutput=False)
    wat = nc.declare_dram_parameter("wat", [128, KH, 128], F32, isOutput=False)
    baf = nc.declare_dram_parameter("baf", [HS, 1], F32, isOutput=False)
    enct = nc.declare_dram_parameter("enct", [128, 16, 128], F32, isOutput=False)
    encc = nc.declare_dram_parameter("encc", [128, 16, 128], F32, isOutput=False)
    woh = nc.declare_dram_parameter("woh", [NVT, 128, KH, VT], F8, isOutput=False)
    woc = nc.declare_dram_parameter("woc", [NVT, 128, KH, VT], F8, isOutput=False)
    bo = nc.declare_dram_parameter("bo", [128, LT], F32, isOutput=False)

    # ---- outputs ----
    logits_out = nc.declare_dram_parameter("logits_out", [VPC], F32, isOutput=True)
    hidden_out = nc.declare_dram_parameter("hidden_out", [2, H], F32, isOutput=True)
    ctx_out = nc.declare_dram_parameter("ctx_out", [H], F32, isOutput=True)
    attn_out = nc.declare_dram_parameter("attn_out", [128, 16], F32, isOutput=True)

    with tile.TileContext(nc) as tc:
        with (
            tc.tile_pool(name="dram", bufs=1, space="DRAM") as dram,
            tc.tile_pool(name="cw", bufs=1) as cw,
            tc.tile_pool(name="sb", bufs=1) as sb,
            tc.tile_pool(name="wph", bufs=NVT) as wph,
            tc.tile_pool(name="wpc", bufs=NVT) as wpc,
            tc.tile_pool(name="pmm", bufs=2, space="PSUM") as pmm,
            tc.tile_pool(name="pvec", bufs=2, space="PSUM") as pvec,
            tc.tile_pool(name="plog", bufs=4, space="PSUM") as plog,
        ):
            # ---- DRAM bounce buffers for collectives ----
            ag1_in = dram.tile([HS], F32)
            ag1_out = dram.tile([H], F32, addr_space="Shared")
            ag2_in = dram.tile([HS], F32)
            ag2_out = dram.tile([H], F32, addr_space="Shared")
            ag3_in = dram.tile([HS], F32)
            ag3_out = dram.tile([H], F32, addr_space="Shared")
            ar4_in = dram.tile([128, 16], F32)
            ar4_out = dram.tile([128, 16], F32, addr_space="Shared")
            ag5_in = dram.tile([HS], F32)
            ag5_out = dram.tile([H], F32, addr_space="Shared")
            ag6_in = dram.tile([2], F32)
            ag6_out = dram.tile([2 * NC_], F32, addr_space="Shared")
            l_dram = dram.tile([VPC], F32)

            # ---- phase 0 on gpsimd: embed row -> AG1, issued first ----
            nc.gpsimd.dma_start(ag1_in[:], emb_f.ap()[idx, :])
            nc.gpsimd.collective_compute(
                "AllGather", mybir.AluOpType.bypass, replica_groups=rg,
                ins=[ag1_in.opt()], outs=[ag1_out.opt()])

            # ---- preloads: big weights on sync HW queues, in need-order ----
            whh0_sb = cw.tile([128, KH * 384], BF16)
            nc.sync.dma_start(
                whh0_sb[:].rearrange("p (k m) -> p k m", m=384), whh0.ap())
            wih0_sb = cw.tile([128, KX * 384], BF16)
            nc.sync.dma_start(
                wih0_sb[:].rearrange("p (k m) -> p k m", m=384), wih0.ap())
            whh1_sb = cw.tile([128, KH * 384], BF16)
            nc.sync.dma_start(
                whh1_sb[:].rearrange("p (k m) -> p k m", m=384), whh1.ap())
            wih1_sb = cw.tile([128, KH * 384], BF16)
            nc.sync.dma_start(
                wih1_sb[:].rearrange("p (k m) -> p k m", m=384), wih1.ap())
            wat_sb = cw.tile([128, KH * 128], F32)
            nc.sync.dma_start(
                wat_sb[:].rearrange("p (k m) -> p k m", m=128), wat.ap())
            enct_sb = cw.tile([128, 16 * 128], F32)
            nc.sync.dma_start(
                enct_sb[:].rearrange("p (k m) -> p k m", m=128), enct.ap())
            encc_sb = cw.tile([128, 16 * 128], F32)
            nc.sync.dma_start(
                encc_sb[:].rearrange("p (k m) -> p k m", m=128), encc.ap())

            # small chain-critical loads on gpsimd SWDGE (separate queues,
            # never stuck behind multi-MB HW-queue transfers)
            xc_sb = cw.tile([128, KH], F32)
            nc.gpsimd.dma_start(xc_sb[:], ctx_in.ap())
            hp0_sb = cw.tile([128, KH], F32)
            nc.gpsimd.dma_start(hp0_sb[:], hp0.ap())
            hp1_sb = cw.tile([128, KH], F32)
            nc.gpsimd.dma_start(hp1_sb[:], hp1.ap())
            bih0_sb = cw.tile([128, 3], F32)
            nc.gpsimd.dma_start(bih0_sb[:], bih0.ap())
            bhh0_sb = cw.tile([128, 3], F32)
            nc.gpsimd.dma_start(bhh0_sb[:], bhh0.ap())
            bih1_sb = cw.tile([128, 3], F32)
            nc.gpsimd.dma_start(bih1_sb[:], bih1.ap())
            bhh1_sb = cw.tile([128, 3], F32)
            nc.gpsimd.dma_start(bhh1_sb[:], bhh1.ap())
            hp0f_sb = cw.tile([128, 1], F32)
            nc.gpsimd.dma_start(hp0f_sb[:], hp0f.ap())
            hp1f_sb = cw.tile([128, 1], F32)
            nc.gpsimd.dma_start(hp1f_sb[:], hp1f.ap())
            baf_sb = cw.tile([128, 1], F32)
            nc.gpsimd.dma_start(baf_sb[:], baf.ap())
            bo_sb = cw.tile([128, LT], F32)
            nc.gpsimd.dma_start(bo_sb[:], bo.ap())

            # ---- W_out stream on sync: h1-half tiles first, then ctx ----
            wth, wtc = [], []
            for j in range(NVT):
                wt = wph.tile([128, KH * VT], F8, tag="wth", name=f"wth{j}")
                nc.sync.dma_start(
                    wt[:].rearrange("p (k v) -> p k v", v=VT), woh.ap()[j])
                wth.append(wt)
            for j in range(NVT):
                wt = wpc.tile([128, KH * VT], F8, tag="wtc", name=f"wtc{j}")
                nc.sync.dma_start(
                    wt[:].rearrange("p (k v) -> p k v", v=VT), woc.ap()[j])
                wtc.append(wt)

            # ---- casts ----
            xc_bf = sb.tile([128, KH], BF16)
            nc.vector.tensor_copy(xc_bf[:], xc_sb[:])
            hp0_bf = sb.tile([128, KH], BF16)
            nc.vector.tensor_copy(hp0_bf[:], hp0_sb[:])
            hp1_bf = sb.tile([128, KH], BF16)
            nc.vector.tensor_copy(hp1_bf[:], hp1_sb[:])

            def gru_matvec(dst_sb, w_sb, rhs_list, nk):
                # one accumulation group per dedicated PSUM tile (interleaved
                # groups sharing a bank lose chunks to sibling start-clears)
                for g in range(3):
                    pg = pmm.tile([128, 1], F32, tag="g", name=f"pg{g}")
                    for kc in range(nk):
                        rhs_bf, col = rhs_list[kc]
                        nc.tensor.matmul(
                            pg[:],
                            w_sb[:, kc * 384 + g * 128: kc * 384 + (g + 1) * 128],
                            rhs_bf[:, col:col + 1],
                            start=(kc == 0), stop=(kc == nk - 1))
                    nc.vector.tensor_copy(dst_sb[:, g:g + 1], pg[:])

            def gru_gates(pi, ph, bih_sb, bhh_sb, hpf_sb, name):
                g1 = sb.tile([128, 3], F32, tag="g1")
                nc.vector.tensor_add(g1[:], pi[:], bih_sb[:])
                g2 = sb.tile([128, 3], F32, tag="g2")
                nc.vector.tensor_add(g2[:], ph[:], bhh_sb[:])
                rzp = sb.tile([128, 2], F32, tag="rzp")
                nc.vector.tensor_add(rzp[:], g1[:, 0:2], g2[:, 0:2])
                rz = sb.tile([128, 2], F32, tag="rz")
                nc.scalar.activation(rz[:], rzp[:], AF.Sigmoid)
                t4 = sb.tile([128, 1], F32, tag="t4")
                nc.vector.tensor_mul(t4[:], rz[:, 0:1], g2[:, 2:3])
                t5 = sb.tile([128, 1], F32, tag="t5")
                nc.vector.tensor_add(t5[:], g1[:, 2:3], t4[:])
                n = sb.tile([128, 1], F32, tag="n")
                nc.scalar.activation(n[:], t5[:], AF.Tanh)
                t6 = sb.tile([128, 1], F32, tag="t6")
                nc.vector.tensor_sub(t6[:], hpf_sb[:], n[:])
                t7 = sb.tile([128, 1], F32, tag="t7")
                nc.vector.tensor_mul(t7[:], rz[:, 1:2], t6[:])
                h = sb.tile([128, 1], F32, tag=name, name=name)
                nc.vector.tensor_add(h[:], n[:], t7[:])
                return h

            # ---- h-path matvecs (no chain dependency; run during AG1) ----
            ph0 = sb.tile([128, 3], F32, tag="ph")
            gru_matvec(ph0, whh0_sb, [(hp0_bf, k) for k in range(KH)], KH)
            ph1 = sb.tile([128, 3], F32, tag="ph1")
            gru_matvec(ph1, whh1_sb, [(hp1_bf, k) for k in range(KH)], KH)

            # ---- layer 0: ctx-half first (overlaps AG1), embed-half after ----
            xe_sb = sb.tile([128, KH], F32)
            nc.gpsimd.dma_start(xe_sb[:], ag1_out.rearrange("(p k) -> p k", k=KH))
            xe_bf = sb.tile([128, KH], BF16)
            nc.vector.tensor_copy(xe_bf[:], xe_sb[:])
            rhs0 = [(xc_bf, k) for k in range(KH)] + [(xe_bf, k) for k in range(KH)]
            pi0 = sb.tile([128, 3], F32, tag="pi")
            gru_matvec(pi0, wih0_sb, rhs0, KX)
            h0_loc = gru_gates(pi0, ph0, bih0_sb, bhh0_sb, hp0f_sb, "h0loc")
            nc.gpsimd.dma_start(ag2_in[:], h0_loc[:])
            nc.gpsimd.collective_compute(
                "AllGather", mybir.AluOpType.bypass, replica_groups=rg,
                ins=[ag2_in.opt()], outs=[ag2_out.opt()])

            # ---- layer 1 ----
            x1_sb = sb.tile([128, KH], F32)
            nc.gpsimd.dma_start(x1_sb[:], ag2_out.rearrange("(p k) -> p k", k=KH))
            x1_bf = sb.tile([128, KH], BF16)
            nc.vector.tensor_copy(x1_bf[:], x1_sb[:])
            pi1 = sb.tile([128, 3], F32, tag="pi1")
            gru_matvec(pi1, wih1_sb, [(x1_bf, k) for k in range(KH)], KH)
            h1_loc = gru_gates(pi1, ph1, bih1_sb, bhh1_sb, hp1f_sb, "h1loc")
            nc.gpsimd.dma_start(ag3_in[:], h1_loc[:])
            nc.gpsimd.collective_compute(
                "AllGather", mybir.AluOpType.bypass, replica_groups=rg,
                ins=[ag3_in.opt()], outs=[ag3_out.opt()])

            # a-vector h1 half: a[k*128+p] = h1 -> [p, k]
            ah_sb = sb.tile([128, KH], F32)
            nc.gpsimd.dma_start(ah_sb[:],
                                ag3_out.rearrange("(k p) -> p k", p=128))
            ah_f8 = sb.tile([128, KH], F8)
            nc.scalar.mul(ah_f8[:], ah_sb[:], SCALE_A)

            # ---- attention: u_local = (Wa.T @ h1)[f_c] ----
            h1f_sb = sb.tile([128, KH], F32)
            nc.gpsimd.dma_start(h1f_sb[:], ag3_out.rearrange("(p k) -> p k", k=KH))
            pu = pvec.tile([128, 1], F32, tag="v")
            for kc in range(KH):
                nc.tensor.matmul(
                    pu[:], wat_sb[:, kc * 128:(kc + 1) * 128],
                    h1f_sb[:, kc:kc + 1], start=(kc == 0), stop=(kc == KH - 1))
            u_sb = sb.tile([128, 1], F32)
            nc.vector.tensor_copy(u_sb[:], pu[:])

            # c0 = (ba . h1) partial over local features, broadcast to partitions
            bh_sb = sb.tile([128, 1], F32)
            nc.vector.tensor_mul(bh_sb[:], baf_sb[:], h1_loc[:])
            c0_sb = sb.tile([128, 1], F32)
            nc.gpsimd.partition_all_reduce(c0_sb[:], bh_sb[:], 128,
                                           bass_isa.ReduceOp.add)

            # partial scores over local feature slice: [128, 16] (s = sc*128 + p)
            ps = pvec.tile([128, 16], F32, tag="v")
            for sc in range(16):
                nc.tensor.matmul(
                    ps[:, sc:sc + 1], enct_sb[:, sc * 128:(sc + 1) * 128],
                    u_sb[:], start=True, stop=True)
            scores_sb = sb.tile([128, 16], F32)
            nc.vector.tensor_scalar_add(scores_sb[:], ps[:], c0_sb[:])
            nc.gpsimd.dma_start(ar4_in[:], scores_sb[:])
            nc.gpsimd.collective_compute(
                "AllReduce", mybir.AluOpType.add, replica_groups=rg,
                ins=[ar4_in.opt()], outs=[ar4_out.opt()])

            # ---- logits h1-half: runs on PE while softmax happens ----
            N_INLINE = 6
            lacc, pls = [], []
            for j in range(NVT):
                pl = plog.tile([1, VT], F32, tag="l", name=f"plh{j}")
                for kc in range(KH):
                    nc.tensor.matmul(
                        pl[:], ah_f8[:, kc:kc + 1],
                        wth[j][:, kc * VT:(kc + 1) * VT],
                        start=(kc == 0), stop=(kc == KH - 1))
                la = cw.tile([1, VT], F32, name=f"lacc{j}")
                lacc.append(la)
                pls.append(pl)
                if j < N_INLINE:
                    nc.vector.tensor_scalar_mul(la[:], pl[:], INV_SCALE)

            # ---- softmax over full scores (replicated per core) ----
            sf_sb = sb.tile([128, 16], F32)
            nc.gpsimd.dma_start(sf_sb[:], ar4_out[:])
            mrow = sb.tile([128, 1], F32)
            nc.vector.reduce_max(mrow[:], sf_sb[:], axis=X)
            mall = sb.tile([128, 1], F32)
            nc.gpsimd.partition_all_reduce(mall[:], mrow[:], 128,
                                           bass_isa.ReduceOp.max)
            negm_sb = sb.tile([128, 1], F32)
            nc.scalar.mul(negm_sb[:], mall[:], -1.0)
            attn_e = sb.tile([128, 16], F32)
            srow = sb.tile([128, 1], F32)
            nc.scalar.activation(attn_e[:], sf_sb[:], AF.Exp, bias=negm_sb[:],
                                 accum_out=srow[:])
            zall = sb.tile([128, 1], F32)
            nc.gpsimd.partition_all_reduce(zall[:], srow[:], 128,
                                           bass_isa.ReduceOp.add)
            rz_sb = sb.tile([128, 1], F32)
            nc.vector.reciprocal(rz_sb[:], zall[:])
            attn_sb = sb.tile([128, 16], F32)
            nc.vector.tensor_scalar_mul(attn_sb[:], attn_e[:], rz_sb[:])

            # flush deferred h1-half descale copies (before ctx copy on DVE,
            # else PE<->DVE deadlock: h1h slots need these, ctx copy needs PE)
            for j in range(N_INLINE, NVT):
                nc.vector.tensor_scalar_mul(lacc[j][:], pls[j][:], INV_SCALE)

            # ---- context slice: ctx[f_c] = sum_s attn[s] * enc[s, f_c] ----
            pctx = pvec.tile([128, 1], F32, tag="v")
            for sc in range(16):
                nc.tensor.matmul(
                    pctx[:], encc_sb[:, sc * 128:(sc + 1) * 128],
                    attn_sb[:, sc:sc + 1], start=(sc == 0), stop=(sc == 15))
            ctxl_sb = sb.tile([128, 1], F32)
            nc.vector.tensor_copy(ctxl_sb[:], pctx[:])
            nc.gpsimd.dma_start(ag5_in[:], ctxl_sb[:])
            nc.gpsimd.collective_compute(
                "AllGather", mybir.AluOpType.bypass, replica_groups=rg,
                ins=[ag5_in.opt()], outs=[ag5_out.opt()])

            # a-vector ctx half
            ac_sb = sb.tile([128, KH], F32)
            nc.gpsimd.dma_start(ac_sb[:],
                                ag5_out.rearrange("(k p) -> p k", p=128))
            ac_f8 = sb.tile([128, KH], F8)
            nc.scalar.mul(ac_f8[:], ac_sb[:], SCALE_A)

            # ---- logits ctx-half + accumulate h1-half ----
            for j in range(NVT):
                pl = plog.tile([1, VT], F32, tag="l")
                for kc in range(KH):
                    nc.tensor.matmul(
                        pl[:], ac_f8[:, kc:kc + 1],
                        wtc[j][:, kc * VT:(kc + 1) * VT],
                        start=(kc == 0), stop=(kc == KH - 1))
                lrow = sb.tile([1, VT], F32, tag="lrow")
                nc.vector.scalar_tensor_tensor(
                    lrow[:], pl[:], INV_SCALE, lacc[j][:],
                    op0=mybir.AluOpType.mult, op1=mybir.AluOpType.add)
                nc.gpsimd.dma_start(l_dram[j * VT:(j + 1) * VT], lrow[:])

            # ---- local log-softmax stats ----
            lg_sb = sb.tile([128, LT], F32)
            nc.gpsimd.dma_start(lg_sb[:], l_dram.rearrange("(p t) -> p t", t=LT))
            nc.vector.tensor_add(lg_sb[:], lg_sb[:], bo_sb[:])
            lmax = sb.tile([128, 1], F32)
            nc.vector.reduce_max(lmax[:], lg_sb[:], axis=X)
            lmall = sb.tile([128, 1], F32)
            nc.gpsimd.partition_all_reduce(lmall[:], lmax[:], 128,
                                           bass_isa.ReduceOp.max)
            negml = sb.tile([128, 1], F32)
            nc.scalar.mul(negml[:], lmall[:], -1.0)
            el = sb.tile([128, LT], F32)
            zrow = sb.tile([128, 1], F32)
            nc.scalar.activation(el[:], lg_sb[:], AF.Exp, bias=negml[:],
                                 accum_out=zrow[:])
            z2all = sb.tile([128, 1], F32)
            nc.gpsimd.partition_all_reduce(z2all[:], zrow[:], 128,
                                           bass_isa.ReduceOp.add)
            stats_sb = sb.tile([1, 2], F32)
            nc.vector.tensor_copy(stats_sb[:, 0:1], lmall[0:1, :])
            nc.vector.tensor_copy(stats_sb[:, 1:2], z2all[0:1, :])
            nc.gpsimd.dma_start(ag6_in[:], stats_sb[:])
            nc.gpsimd.collective_compute(
                "AllGather", mybir.AluOpType.bypass, replica_groups=rg,
                ins=[ag6_in.opt()], outs=[ag6_out.opt()])

            # ---- global normalization ----
            g_sb = sb.tile([NC_, 2], F32)
            nc.gpsimd.dma_start(g_sb[:], ag6_out.rearrange("(c t) -> c t", t=2))
            gmall = sb.tile([NC_, 1], F32)
            nc.gpsimd.partition_all_reduce(gmall[:], g_sb[:, 0:1], NC_,
                                           bass_isa.ReduceOp.max)
            negM8 = sb.tile([NC_, 1], F32)
            nc.scalar.mul(negM8[:], gmall[:], -1.0)
            ee = sb.tile([NC_, 1], F32)
            nc.scalar.activation(ee[:], g_sb[:, 0:1], AF.Exp, bias=negM8[:])
            zz = sb.tile([NC_, 1], F32)
            nc.vector.tensor_mul(zz[:], ee[:], g_sb[:, 1:2])
            zzall = sb.tile([NC_, 1], F32)
            nc.gpsimd.partition_all_reduce(zzall[:], zz[:], NC_,
                                           bass_isa.ReduceOp.add)
            lnz = sb.tile([1, 1], F32)
            nc.scalar.activation(lnz[:], zzall[0:1, :], AF.Ln)
            tot = sb.tile([1, 1], F32)
            nc.vector.tensor_add(tot[:], lnz[:], gmall[0:1, :])
            totb = sb.tile([128, 1], F32)
            nc.gpsimd.partition_broadcast(totb[:], tot[:], 128)
            out_sb = sb.tile([128, LT], F32)
            nc.vector.tensor_scalar_sub(out_sb[:], lg_sb[:], totb[:])
            nc.gpsimd.dma_start(
                logits_out.ap().rearrange("(p t) -> p t", t=LT), out_sb[:])

            # deferred small outputs (off the critical chain)
            nc.gpsimd.dma_start(attn_out.ap(), attn_sb[:])
            nc.gpsimd.dma_start(
                hidden_out.ap()[0, :].rearrange("(p k) -> p k", k=KH), x1_sb[:])
            nc.gpsimd.dma_start(
                hidden_out.ap()[1, :].rearrange("(p k) -> p k", k=KH), h1f_sb[:])
            nc.gpsimd.dma_start(
                ctx_out.ap().rearrange("(k p) -> p k", p=128), ac_sb[:])

    nc.compile()
    return nc


def prep_inputs(word_input, last_context, last_hidden, encoder_outputs,
                emb, W_ih0, W_hh0, b_ih0, b_hh0, W_ih1, W_hh1, b_ih1, b_hh1,
                Wa, ba, W_out, b_out):
    f32 = np.float32
    idx = int(np.asarray(word_input).reshape(-1)[0])
    emb = np.asarray(emb, f32)
    enc = np.asarray(encoder_outputs, f32)[:, 0, :]           # [S, H]
    ctx = np.asarray(last_context, f32).reshape(-1)           # [H]
    hp0_np = np.asarray(last_hidden, f32)[0, 0]               # [H]
    hp1_np = np.asarray(last_hidden, f32)[1, 0]
    Wp = np.zeros((VP, 2 * H), f32)
    Wp[:V] = np.asarray(W_out, f32)
    bp = np.full((VP,), -1e30, f32)
    bp[:V] = np.asarray(b_out, f32)

    def gate_rows(Wm):
        Wm = np.asarray(Wm, f32)
        return Wm.reshape(3, H, Wm.shape[1])                  # [3, H, in]

    Wi0, Wh0 = gate_rows(W_ih0), gate_rows(W_hh0)
    Wi1, Wh1 = gate_rows(W_ih1), gate_rows(W_hh1)

    def bias3(b):
        return np.asarray(b, f32).reshape(3, H)

    bi0, bh0_, bi1, bh1_ = bias3(b_ih0), bias3(b_hh0), bias3(b_ih1), bias3(b_hh1)
    Wa_np = np.asarray(Wa, f32)
    ba_np = np.asarray(ba, f32)

    in_maps = []
    for c in range(NC_):
        f = slice(c * HS, (c + 1) * HS)

        def gshard(W3, nk):
            # p-major: out[p, k, g*128+j] = W[g, c*128+j, in-col p*nk+k]
            sub = np.concatenate([W3[0, f], W3[1, f], W3[2, f]], axis=0)
            return np.ascontiguousarray(
                sub.T.reshape(128, nk, 384)).astype(BF16NP)

        # layer-0 ih: chunks 0..7 = ctx half, 8..15 = embed half
        sub0 = np.concatenate([Wi0[0, f], Wi0[1, f], Wi0[2, f]], axis=0)
        sub0T = sub0.T                                         # [2048, 384]
        wih0_h = np.concatenate([
            sub0T[H:].reshape(128, KH, 384),                   # ctx cols
            sub0T[:H].reshape(128, KH, 384),                   # emb cols
        ], axis=1).astype(BF16NP)

        E = np.ascontiguousarray(enc[:, f])                    # [S, 128]
        Wc = Wp[c * VPC:(c + 1) * VPC]                         # [VPC, 2H]
        m = {
            "emb_f": np.ascontiguousarray(emb[:, f]),
            "ctx_in": ctx.reshape(128, KH).copy(),
            "hp0": hp0_np.reshape(128, KH).copy(),
            "hp1": hp1_np.reshape(128, KH).copy(),
            "hp0f": np.ascontiguousarray(hp0_np[f]).reshape(HS, 1),
            "hp1f": np.ascontiguousarray(hp1_np[f]).reshape(HS, 1),
            "wih0": np.ascontiguousarray(wih0_h),
            "whh0": gshard(Wh0, KH),
            "wih1": gshard(Wi1, KH),
            "whh1": gshard(Wh1, KH),
            "bih0": np.ascontiguousarray(bi0[:, f].T),
            "bhh0": np.ascontiguousarray(bh0_[:, f].T),
            "bih1": np.ascontiguousarray(bi1[:, f].T),
            "bhh1": np.ascontiguousarray(bh1_[:, f].T),
            "wat": np.ascontiguousarray(Wa_np[:, f].reshape(128, KH, 128)),
            "baf": np.ascontiguousarray(ba_np[f]).reshape(HS, 1),
            "enct": np.ascontiguousarray(E.T.reshape(128, 16, 128)),
            "encc": np.ascontiguousarray(
                E.reshape(16, 128, 128).transpose(1, 0, 2)),
            "woh": np.ascontiguousarray(
                Wc.T[:H].reshape(KH, 128, NVT, VT).transpose(2, 1, 0, 3)
                * SCALE_W).astype(F8NP),
            "woc": np.ascontiguousarray(
                Wc.T[H:].reshape(KH, 128, NVT, VT).transpose(2, 1, 0, 3)
                * SCALE_W).astype(F8NP),
            "bo": np.ascontiguousarray(
                bp[c * VPC:(c + 1) * VPC].reshape(128, LT)),
        }
        in_maps.append(m)
    return idx, in_maps


def assemble_outputs(results):
    logits = np.concatenate(
        [results[c]["logits_out"] for c in range(NC_)])[:V].reshape(1, V)
    context = results[0]["ctx_out"].reshape(1, H).astype(np.float32)
    hidden = results[0]["hidden_out"].reshape(2, 1, H).astype(np.float32)
    attn = np.ascontiguousarray(
        results[0]["attn_out"].T).reshape(1, 1, S).astype(np.float32)
    return (logits.astype(np.float32), context, hidden, attn)


def run(inputs: dict, trace: bool = False, w_bufs: int = 8):
    idx, in_maps = prep_inputs(**inputs)
    nc = build_graph(idx, w_bufs=w_bufs)
    res = run_bass_kernel_spmd(nc, in_maps, list(range(NC_)), trace=trace)
    return assemble_outputs(res.results), res


def kernel(**inputs):
    out, _ = run(inputs, trace=False)
    return out


# revision 15
# speedup vs baseline: 1.0216x; 1.0216x over previous
"""AttnDecoderRNN single-step decoder on 8 TRN2 NeuronCores.

Sharding: hidden features (H=1024 -> 128/core) for emb/GRU/attention;
vocab rows (50257 -> 6656 padded/core) for W_out logits + log_softmax.
All cores run one SPMD graph; per-core differences come via input data.

Queues: sync = weight preloads + W_out stream only (no chain waits);
gpsimd = collectives + all chain/epilogue DMAs (serial anyway).
"""
import numpy as np
import ml_dtypes

import concourse.bass as bass
import concourse.bacc as bacc
import concourse.mybir as mybir
from concourse import tile
from concourse import bass_isa
from concourse.bass_utils import run_bass_kernel_spmd

F32 = mybir.dt.float32
BF16 = mybir.dt.bfloat16
BF16NP = ml_dtypes.bfloat16
F8 = mybir.dt.float8e4
F8NP = ml_dtypes.float8_e4m3
SCALE_W = 64.0
SCALE_A = 16.0
INV_SCALE = 1.0 / (SCALE_W * SCALE_A)
AF = mybir.ActivationFunctionType
X = mybir.AxisListType.X

NC_ = 8
V, H, S = 50257, 1024, 2048
HS = H // NC_            # 128 features per core
KX = 16                  # x0 chunks (2048/128)
KH = 8                   # h chunks (1024/128)
VT = 512                 # vocab tile free dim
NVT = 13                 # vocab tiles per core
VPC = VT * NVT           # 6656 padded vocab rows per core
VP = VPC * NC_           # 53248
LT = VPC // 128          # 52 logits per partition in [128, 52] layout


def build_graph(idx: int, w_bufs: int = 8, debug: bool = False):
    nc = bacc.Bacc("TRN2", debug=False, num_devices=NC_)
    rg = [list(range(NC_))]

    # ---- inputs (p-major layouts so every DMA row is contiguous) ----
    emb_f = nc.declare_dram_parameter("emb_f", [V, HS], F32, isOutput=False)
    ctx_in = nc.declare_dram_parameter("ctx_in", [128, KH], F32, isOutput=False)
    hp0 = nc.declare_dram_parameter("hp0", [128, KH], F32, isOutput=False)
    hp1 = nc.declare_dram_parameter("hp1", [128, KH], F32, isOutput=False)
    hp0f = nc.declare_dram_parameter("hp0f", [HS, 1], F32, isOutput=False)
    hp1f = nc.declare_dram_parameter("hp1f", [HS, 1], F32, isOutput=False)
    wih0 = nc.declare_dram_parameter("wih0", [128, KX, 384], BF16, isOutput=False)
    whh0 = nc.declare_dram_parameter("whh0", [128, KH, 384], BF16, isOutput=False)
    wih1 = nc.declare_dram_parameter("wih1", [128, KH, 384], BF16, isOutput=False)
    whh1 = nc.declare_dram_parameter("whh1", [128, KH, 384], BF16, isOutput=False)
    bih0 = nc.declare_dram_parameter("bih0", [128, 3], F32, isOutput=False)
    bhh0 = nc.declare_dram_parameter("bhh0", [128, 3], F32, isOutput=False)
    bih1 = nc.declare_dram_parameter("bih1", [128, 3], F32, isOutput=False)
    bhh1 = nc.declare_dram_parameter("bhh1", [128, 3], F32, isOutput=False)
    wat = nc.declare_dram_parameter("wat", [128, KH, 128], F32, isOutput=False)
    baf = nc.declare_dram_parameter("baf", [HS, 1], F32, isOutput=False)
    enct = nc.declare_dram_parameter("enct", [128, 16, 128], F32, isOutput=False)
    encc = nc.declare_dram_parameter("encc", [128, 16, 128], F32, isOutput=False)
    woh = nc.declare_dram_parameter("woh", [NVT, 128, KH, VT], F8, isOutput=False)
    woc = nc.declare_dram_parameter("woc", [NVT, 128, KH, VT], F8, isOutput=False)
    bo = nc.declare_dram_parameter("bo", [128, LT], F32, isOutput=False)

    # ---- outputs ----
    logits_out = nc.declare_dram_parameter("logits_out", [VPC], F32, isOutput=True)
    hidden_out = nc.declare_dram_parameter("hidden_out", [2, H], F32, isOutput=True)
    ctx_out = nc.declare_dram_parameter("ctx_out", [H], F32, isOutput=True)
    attn_out = nc.declare_dram_parameter("attn_out", [128, 16], F32, isOutput=True)

    with tile.TileContext(nc) as tc:
        with (
            tc.tile_pool(name="dram", bufs=1, space="DRAM") as dram,
            tc.tile_pool(name="cw", bufs=1) as cw,
            tc.tile_pool(name="sb", bufs=1) as sb,
            tc.tile_pool(name="wph", bufs=NVT) as wph,
            tc.tile_pool(name="wpc", bufs=NVT) as wpc,
            tc.tile_pool(name="pmm", bufs=2, space="PSUM") as pmm,
            tc.tile_pool(name="pvec", bufs=2, space="PSUM") as pvec,
            tc.tile_pool(name="plog", bufs=4, space="PSUM") as plog,
        ):
            # ---- DRAM bounce buffers for collectives ----
            ag1_in = dram.tile([HS], F32)
            ag1_out = dram.tile([H], F32, addr_space="Shared")
            ag2_in = dram.tile([HS], F32)
            ag2_out = dram.tile([H], F32, addr_space="Shared")
            ag3_in = dram.tile([HS], F32)
            ag3_out = dram.tile([H], F32, addr_space="Shared")
            ar4_in = dram.tile([128, 16], F32)
            ar4_out = dram.tile([128, 16], F32, addr_space="Shared")
            ag5_in = dram.tile([HS], F32)
            ag5_out = dram.tile([H], F32, addr_space="Shared")
            ag6_in = dram.tile([2], F32)
            ag6_out = dram.tile([2 * NC_], F32, addr_space="Shared")
            l_dram = dram.tile([VPC], F32)

            # ---- phase 0 on gpsimd: embed row -> AG1, issued first ----
            nc.gpsimd.dma_start(ag1_in[:], emb_f.ap()[idx, :])
            nc.gpsimd.collective_compute(
                "AllGather", mybir.AluOpType.bypass, replica_groups=rg,
                ins=[ag1_in.opt()], outs=[ag1_out.opt()])

            # ---- preloads: big weights on sync HW queues, in need-order ----
            whh0_sb = cw.tile([128, KH * 384], BF16)
            nc.sync.dma_start(
                whh0_sb[:].rearrange("p (k m) -> p k m", m=384), whh0.ap())
            wih0_sb = cw.tile([128, KX * 384], BF16)
            nc.sync.dma_start(
                wih0_sb[:].rearrange("p (k m) -> p k m", m=384), wih0.ap())
            whh1_sb = cw.tile([128, KH * 384], BF16)
            nc.sync.dma_start(
                whh1_sb[:].rearrange("p (k m) -> p k m", m=384), whh1.ap())
            wih1_sb = cw.tile([128, KH * 384], BF16)
            nc.sync.dma_start(
                wih1_sb[:].rearrange("p (k m) -> p k m", m=384), wih1.ap())
            wat_sb = cw.tile([128, KH * 128], F32)
            nc.sync.dma_start(
                wat_sb[:].rearrange("p (k m) -> p k m", m=128), wat.ap())
            enct_sb = cw.tile([128, 16 * 128], F32)
            nc.sync.dma_start(
                enct_sb[:].rearrange("p (k m) -> p k m", m=128), enct.ap())
            encc_sb = cw.tile([128, 16 * 128], F32)
            nc.sync.dma_start(
                encc_sb[:].rearrange("p (k m) -> p k m", m=128), encc.ap())

            # small chain-critical loads on gpsimd SWDGE (separate queues,
            # never stuck behind multi-MB HW-queue transfers)
            xc_sb = cw.tile([128, KH], F32)
            nc.gpsimd.dma_start(xc_sb[:], ctx_in.ap())
            hp0_sb = cw.tile([128, KH], F32)
            nc.gpsimd.dma_start(hp0_sb[:], hp0.ap())
            hp1_sb = cw.tile([128, KH], F32)
            nc.gpsimd.dma_start(hp1_sb[:], hp1.ap())
            bih0_sb = cw.tile([128, 3], F32)
            nc.gpsimd.dma_start(bih0_sb[:], bih0.ap())
            bhh0_sb = cw.tile([128, 3], F32)
            nc.gpsimd.dma_start(bhh0_sb[:], bhh0.ap())
            bih1_sb = cw.tile([128, 3], F32)
            nc.gpsimd.dma_start(bih1_sb[:], bih1.ap())
            bhh1_sb = cw.tile([128, 3], F32)
            nc.gpsimd.dma_start(bhh1_sb[:], bhh1.ap())
            hp0f_sb = cw.tile([128, 1], F32)
            nc.gpsimd.dma_start(hp0f_sb[:], hp0f.ap())
            hp1f_sb = cw.tile([128, 1], F32)
            nc.gpsimd.dma_start(hp1f_sb[:], hp1f.ap())
            baf_sb = cw.tile([128, 1], F32)
            nc.gpsimd.dma_start(baf_sb[:], baf.ap())
            bo_sb = cw.tile([128, LT], F32)
            nc.gpsimd.dma_start(bo_sb[:], bo.ap())

            # ---- W_out stream on sync: h1-half tiles first, then ctx ----
            wth, wtc = [], []
            for j in range(NVT):
                wt = wph.tile([128, KH * VT], F8, tag="wth", name=f"wth{j}")
                nc.sync.dma_start(
                    wt[:].rearrange("p (k v) -> p k v", v=VT), woh.ap()[j])
                wth.append(wt)
            for j in range(NVT):
                wt = wpc.tile([128, KH * VT], F8, tag="wtc", name=f"wtc{j}")
                nc.sync.dma_start(
                    wt[:].rearrange("p (k v) -> p k v", v=VT), woc.ap()[j])
                wtc.append(wt)

            # ---- casts ----
            xc_bf = sb.tile([128, KH], BF16)
            nc.vector.tensor_copy(xc_bf[:], xc_sb[:])
            hp0_bf = sb.tile([128, KH], BF16)
            nc.vector.tensor_copy(hp0_bf[:], hp0_sb[:])
            hp1_bf = sb.tile([128, KH], BF16)
            nc.vector.tensor_copy(hp1_bf[:], hp1_sb[:])

            def gru_matvec(dst_sb, w_sb, rhs_list, nk):
                # one accumulation group per dedicated PSUM tile (interleaved
                # groups sharing a bank lose chunks to sibling start-clears)
                for g in range(3):
                    pg = pmm.tile([128, 1], F32, tag="g", name=f"pg{g}")
                    for kc in range(nk):
                        rhs_bf, col = rhs_list[kc]
                        nc.tensor.matmul(
                            pg[:],
                            w_sb[:, kc * 384 + g * 128: kc * 384 + (g + 1) * 128],
                            rhs_bf[:, col:col + 1],
                            start=(kc == 0), stop=(kc == nk - 1))
                    nc.vector.tensor_copy(dst_sb[:, g:g + 1], pg[:])

            def gru_gates(pi, ph, bih_sb, bhh_sb, hpf_sb, name):
                g1 = sb.tile([128, 3], F32, tag="g1")
                nc.vector.tensor_add(g1[:], pi[:], bih_sb[:])
                g2 = sb.tile([128, 3], F32, tag="g2")
                nc.vector.tensor_add(g2[:], ph[:], bhh_sb[:])
                rzp = sb.tile([128, 2], F32, tag="rzp")
                nc.vector.tensor_add(rzp[:], g1[:, 0:2], g2[:, 0:2])
                rz = sb.tile([128, 2], F32, tag="rz")
                nc.scalar.activation(rz[:], rzp[:], AF.Sigmoid)
                t4 = sb.tile([128, 1], F32, tag="t4")
                nc.vector.tensor_mul(t4[:], rz[:, 0:1], g2[:, 2:3])
                t5 = sb.tile([128, 1], F32, tag="t5")
                nc.vector.tensor_add(t5[:], g1[:, 2:3], t4[:])
                n = sb.tile([128, 1], F32, tag="n")
                nc.scalar.activation(n[:], t5[:], AF.Tanh)
                t6 = sb.tile([128, 1], F32, tag="t6")
                nc.vector.tensor_sub(t6[:], hpf_sb[:], n[:])
                t7 = sb.tile([128, 1], F32, tag="t7")
                nc.vector.tensor_mul(t7[:], rz[:, 1:2], t6[:])
                h = sb.tile([128, 1], F32, tag=name, name=name)
                nc.vector.tensor_add(h[:], n[:], t7[:])
                return h

            # ---- h-path matvecs (no chain dependency; run during AG1) ----
            ph0 = sb.tile([128, 3], F32, tag="ph")
            gru_matvec(ph0, whh0_sb, [(hp0_bf, k) for k in range(KH)], KH)
            ph1 = sb.tile([128, 3], F32, tag="ph1")
            gru_matvec(ph1, whh1_sb, [(hp1_bf, k) for k in range(KH)], KH)

            # ---- layer 0: ctx-half first (overlaps AG1), embed-half after ----
            xe_sb = sb.tile([128, KH], F32)
            nc.gpsimd.dma_start(xe_sb[:], ag1_out.rearrange("(p k) -> p k", k=KH))
            xe_bf = sb.tile([128, KH], BF16)
            nc.vector.tensor_copy(xe_bf[:], xe_sb[:])
            rhs0 = [(xc_bf, k) for k in range(KH)] + [(xe_bf, k) for k in range(KH)]
            pi0 = sb.tile([128, 3], F32, tag="pi")
            gru_matvec(pi0, wih0_sb, rhs0, KX)
            h0_loc = gru_gates(pi0, ph0, bih0_sb, bhh0_sb, hp0f_sb, "h0loc")
            nc.gpsimd.dma_start(ag2_in[:], h0_loc[:])
            nc.gpsimd.collective_compute(
                "AllGather", mybir.AluOpType.bypass, replica_groups=rg,
                ins=[ag2_in.opt()], outs=[ag2_out.opt()])

            # ---- layer 1 ----
            x1_sb = sb.tile([128, KH], F32)
            nc.gpsimd.dma_start(x1_sb[:], ag2_out.rearrange("(p k) -> p k", k=KH))
            x1_bf = sb.tile([128, KH], BF16)
            nc.vector.tensor_copy(x1_bf[:], x1_sb[:])
            pi1 = sb.tile([128, 3], F32, tag="pi1")
            gru_matvec(pi1, wih1_sb, [(x1_bf, k) for k in range(KH)], KH)
            h1_loc = gru_gates(pi1, ph1, bih1_sb, bhh1_sb, hp1f_sb, "h1loc")
            nc.gpsimd.dma_start(ag3_in[:], h1_loc[:])
            nc.gpsimd.collective_compute(
                "AllGather", mybir.AluOpType.bypass, replica_groups=rg,
                ins=[ag3_in.opt()], outs=[ag3_out.opt()])

            # a-vector h1 half: a[k*128+p] = h1 -> [p, k]
            ah_sb = sb.tile([128, KH], F32)
            nc.gpsimd.dma_start(ah_sb[:],
                                ag3_out.rearrange("(k p) -> p k", p=128))
            ah_f8 = sb.tile([128, KH], F8)
            nc.scalar.mul(ah_f8[:], ah_sb[:], SCALE_A)

            # ---- attention: u_local = (Wa.T @ h1)[f_c] ----
            h1f_sb = sb.tile([128, KH], F32)
            nc.gpsimd.dma_start(h1f_sb[:], ag3_out.rearrange("(p k) -> p k", k=KH))
            pu = pvec.tile([128, 1], F32, tag="v")
            for kc in range(KH):
                nc.tensor.matmul(
                    pu[:], wat_sb[:, kc * 128:(kc + 1) * 128],
                    h1f_sb[:, kc:kc + 1], start=(kc == 0), stop=(kc == KH - 1))
            u_sb = sb.tile([128, 1], F32)
            nc.vector.tensor_copy(u_sb[:], pu[:])

            # c0 = (ba . h1) partial over local features, broadcast to partitions
            bh_sb = sb.tile([128, 1], F32)
            nc.vector.tensor_mul(bh_sb[:], baf_sb[:], h1_loc[:])
            c0_sb = sb.tile([128, 1], F32)
            nc.gpsimd.partition_all_reduce(c0_sb[:], bh_sb[:], 128,
                                           bass_isa.ReduceOp.add)

            # partial scores over local feature slice: [128, 16] (s = sc*128 + p)
            ps = pvec.tile([128, 16], F32, tag="v")
            for sc in range(16):
                nc.tensor.matmul(
                    ps[:, sc:sc + 1], enct_sb[:, sc * 128:(sc + 1) * 128],
                    u_sb[:], start=True, stop=True)
            scores_sb = sb.tile([128, 16], F32)
            nc.vector.tensor_scalar_add(scores_sb[:], ps[:], c0_sb[:])
            nc.gpsimd.dma_start(ar4_in[:], scores_sb[:])
            nc.gpsimd.collective_compute(
                "AllReduce", mybir.AluOpType.add, replica_groups=rg,
                ins=[ar4_in.opt()], outs=[ar4_out.opt()])

            # ---- logits h1-half: runs on PE while softmax happens ----
            N_INLINE = NVT
            lacc, pls = [], []
            for j in range(NVT):
                pl = plog.tile([1, VT], F32, tag="l", name=f"plh{j}")
                for kc in range(KH):
                    nc.tensor.matmul(
                        pl[:], ah_f8[:, kc:kc + 1],
                        wth[j][:, kc * VT:(kc + 1) * VT],
                        start=(kc == 0), stop=(kc == KH - 1))
                la = cw.tile([1, VT], F32, name=f"lacc{j}")
                lacc.append(la)
                pls.append(pl)
                if j < N_INLINE:
                    nc.vector.tensor_scalar_mul(la[:], pl[:], INV_SCALE)

            # ---- softmax over full scores (replicated per core) ----
            sf_sb = sb.tile([128, 16], F32)
            nc.gpsimd.dma_start(sf_sb[:], ar4_out[:])
            mrow = sb.tile([128, 1], F32)
            nc.vector.reduce_max(mrow[:], sf_sb[:], axis=X)
            mall = sb.tile([128, 1], F32)
            nc.gpsimd.partition_all_reduce(mall[:], mrow[:], 128,
                                           bass_isa.ReduceOp.max)
            negm_sb = sb.tile([128, 1], F32)
            nc.scalar.mul(negm_sb[:], mall[:], -1.0)
            attn_e = sb.tile([128, 16], F32)
            srow = sb.tile([128, 1], F32)
            nc.scalar.activation(attn_e[:], sf_sb[:], AF.Exp, bias=negm_sb[:],
                                 accum_out=srow[:])
            zall = sb.tile([128, 1], F32)
            nc.gpsimd.partition_all_reduce(zall[:], srow[:], 128,
                                           bass_isa.ReduceOp.add)
            rz_sb = sb.tile([128, 1], F32)
            nc.vector.reciprocal(rz_sb[:], zall[:])
            attn_sb = sb.tile([128, 16], F32)
            nc.vector.tensor_scalar_mul(attn_sb[:], attn_e[:], rz_sb[:])

            # ---- context slice: ctx[f_c] = sum_s attn[s] * enc[s, f_c] ----
            pctx = pvec.tile([128, 1], F32, tag="v")
            for sc in range(16):
                nc.tensor.matmul(
                    pctx[:], encc_sb[:, sc * 128:(sc + 1) * 128],
                    attn_sb[:, sc:sc + 1], start=(sc == 0), stop=(sc == 15))
            ctxl_sb = sb.tile([128, 1], F32)
            nc.vector.tensor_copy(ctxl_sb[:], pctx[:])
            nc.gpsimd.dma_start(ag5_in[:], ctxl_sb[:])
            nc.gpsimd.collective_compute(
                "AllGather", mybir.AluOpType.bypass, replica_groups=rg,
                ins=[ag5_in.opt()], outs=[ag5_out.opt()])

            # a-vector ctx half
            ac_sb = sb.tile([128, KH], F32)
            nc.gpsimd.dma_start(ac_sb[:],
                                ag5_out.rearrange("(k p) -> p k", p=128))
            ac_f8 = sb.tile([128, KH], F8)
            nc.scalar.mul(ac_f8[:], ac_sb[:], SCALE_A)

            # ---- logits ctx-half + accumulate h1-half ----
            for j in range(NVT):
                pl = plog.tile([1, VT], F32, tag="l")
                for kc in range(KH):
                    nc.tensor.matmul(
                        pl[:], ac_f8[:, kc:kc + 1],
                        wtc[j][:, kc * VT:(kc + 1) * VT],
                        start=(kc == 0), stop=(kc == KH - 1))
                lrow = sb.tile([1, VT], F32, tag="lrow")
                nc.vector.scalar_tensor_tensor(
                    lrow[:], pl[:], INV_SCALE, lacc[j][:],
                    op0=mybir.AluOpType.mult, op1=mybir.AluOpType.add)
                nc.gpsimd.dma_start(l_dram[j * VT:(j + 1) * VT], lrow[:])

            # ---- local log-softmax stats ----
            lg_sb = sb.tile([128, LT], F32)
            nc.gpsimd.dma_start(lg_sb[:], l_dram.rearrange("(p t) -> p t", t=LT))
            nc.vector.tensor_add(lg_sb[:], lg_sb[:], bo_sb[:])
            lmax = sb.tile([128, 1], F32)
            nc.vector.reduce_max(lmax[:], lg_sb[:], axis=X)
            lmall = sb.tile([128, 1], F32)
            nc.gpsimd.partition_all_reduce(lmall[:], lmax[:], 128,
                                           bass_isa.ReduceOp.max)
            negml = sb.tile([128, 1], F32)
            nc.scalar.mul(negml[:], lmall[:], -1.0)
            el = sb.tile([128, LT], F32)
            zrow = sb.tile([128, 1], F32)
            nc.scalar.activation(el[:], lg_sb[:], AF.Exp, bias=negml[:],
                                 accum_out=zrow[:])
            z2all = sb.tile([128, 1], F32)
            nc.gpsimd.partition_all_reduce(z2all[:], zrow[:], 128,
                                           bass_isa.ReduceOp.add)
            stats_sb = sb.tile([1, 2], F32)
            nc.vector.tensor_copy(stats_sb[:, 0:1], lmall[0:1, :])
            nc.vector.tensor_copy(stats_sb[:, 1:2], z2all[0:1, :])
            nc.gpsimd.dma_start(ag6_in[:], stats_sb[:])
            nc.gpsimd.collective_compute(
                "AllGather", mybir.AluOpType.bypass, replica_groups=rg,
                ins=[ag6_in.opt()], outs=[ag6_out.opt()])

            # ---- global normalization ----
            g_sb = sb.tile([NC_, 2], F32)
            nc.gpsimd.dma_start(g_sb[:], ag6_out.rearrange("(c t) -> c t", t=2))
            gmall = sb.tile([NC_, 1], F32)
            nc.gpsimd.partition_all_reduce(gmall[:], g_sb[:, 0:1], NC_,
                                           bass_isa.ReduceOp.max)
            negM8 = sb.tile([NC_, 1], F32)
            nc.scalar.mul(negM8[:], gmall[:], -1.0)
            ee = sb.tile([NC_, 1], F32)
            nc.scalar.activation(ee[:], g_sb[:, 0:1], AF.Exp, bias=negM8[:])
            zz = sb.tile([NC_, 1], F32)
            nc.vector.tensor_mul(zz[:], ee[:], g_sb[:, 1:2])
            zzall = sb.tile([NC_, 1], F32)
            nc.gpsimd.partition_all_reduce(zzall[:], zz[:], NC_,
                                           bass_isa.ReduceOp.add)
            lnz = sb.tile([1, 1], F32)
            nc.scalar.activation(lnz[:], zzall[0:1, :], AF.Ln)
            tot = sb.tile([1, 1], F32)
            nc.vector.tensor_add(tot[:], lnz[:], gmall[0:1, :])
            totb = sb.tile([128, 1], F32)
            nc.gpsimd.partition_broadcast(totb[:], tot[:], 128)
            out_sb = sb.tile([128, LT], F32)
            nc.vector.tensor_scalar_sub(out_sb[:], lg_sb[:], totb[:])
            nc.gpsimd.dma_start(
                logits_out.ap().rearrange("(p t) -> p t", t=LT), out_sb[:])

            # deferred small outputs (off the critical chain)
            nc.gpsimd.dma_start(attn_out.ap(), attn_sb[:])
            nc.gpsimd.dma_start(
                hidden_out.ap()[0, :].rearrange("(p k) -> p k", k=KH), x1_sb[:])
            nc.gpsimd.dma_start(
                hidden_out.ap()[1, :].rearrange("(p k) -> p k", k=KH), h1f_sb[:])
            nc.gpsimd.dma_start(
                ctx_out.ap().rearrange("(k p) -> p k", p=128), ac_sb[:])

    nc.compile()
    return nc


def prep_inputs(word_input, last_context, last_hidden, encoder_outputs,
                emb, W_ih0, W_hh0, b_ih0, b_hh0, W_ih1, W_hh1, b_ih1, b_hh1,
                Wa, ba, W_out, b_out):
    f32 = np.float32
    idx = int(np.asarray(word_input).reshape(-1)[0])
    emb = np.asarray(emb, f32)
    enc = np.asarray(encoder_outputs, f32)[:, 0, :]           # [S, H]
    ctx = np.asarray(last_context, f32).reshape(-1)           # [H]
    hp0_np = np.asarray(last_hidden, f32)[0, 0]               # [H]
    hp1_np = np.asarray(last_hidden, f32)[1, 0]
    Wp = np.zeros((VP, 2 * H), f32)
    Wp[:V] = np.asarray(W_out, f32)
    bp = np.full((VP,), -1e30, f32)
    bp[:V] = np.asarray(b_out, f32)

    def gate_rows(Wm):
        Wm = np.asarray(Wm, f32)
        return Wm.reshape(3, H, Wm.shape[1])                  # [3, H, in]

    Wi0, Wh0 = gate_rows(W_ih0), gate_rows(W_hh0)
    Wi1, Wh1 = gate_rows(W_ih1), gate_rows(W_hh1)

    def bias3(b):
        return np.asarray(b, f32).reshape(3, H)

    bi0, bh0_, bi1, bh1_ = bias3(b_ih0), bias3(b_hh0), bias3(b_ih1), bias3(b_hh1)
    Wa_np = np.asarray(Wa, f32)
    ba_np = np.asarray(ba, f32)

    in_maps = []
    for c in range(NC_):
        f = slice(c * HS, (c + 1) * HS)

        def gshard(W3, nk):
            # p-major: out[p, k, g*128+j] = W[g, c*128+j, in-col p*nk+k]
            sub = np.concatenate([W3[0, f], W3[1, f], W3[2, f]], axis=0)
            return np.ascontiguousarray(
                sub.T.reshape(128, nk, 384)).astype(BF16NP)

        # layer-0 ih: chunks 0..7 = ctx half, 8..15 = embed half
        sub0 = np.concatenate([Wi0[0, f], Wi0[1, f], Wi0[2, f]], axis=0)
        sub0T = sub0.T                                         # [2048, 384]
        wih0_h = np.concatenate([
            sub0T[H:].reshape(128, KH, 384),                   # ctx cols
            sub0T[:H].reshape(128, KH, 384),                   # emb cols
        ], axis=1).astype(BF16NP)

        E = np.ascontiguousarray(enc[:, f])                    # [S, 128]
        Wc = Wp[c * VPC:(c + 1) * VPC]                         # [VPC, 2H]
        m = {
            "emb_f": np.ascontiguousarray(emb[:, f]),
            "ctx_in": ctx.reshape(128, KH).copy(),
            "hp0": hp0_np.reshape(128, KH).copy(),
            "hp1": hp1_np.reshape(128, KH).copy(),
            "hp0f": np.ascontiguousarray(hp0_np[f]).reshape(HS, 1),
            "hp1f": np.ascontiguousarray(hp1_np[f]).reshape(HS, 1),
            "wih0": np.ascontiguousarray(wih0_h),
            "whh0": gshard(Wh0, KH),
            "wih1": gshard(Wi1, KH),
            "whh1": gshard(Wh1, KH),
            "bih0": np.ascontiguousarray(bi0[:, f].T),
            "bhh0": np.ascontiguousarray(bh0_[:, f].T),
            "bih1": np.ascontiguousarray(bi1[:, f].T),
            "bhh1": np.ascontiguousarray(bh1_[:, f].T),
            "wat": np.ascontiguousarray(Wa_np[:, f].reshape(128, KH, 128)),
            "baf": np.ascontiguousarray(ba_np[f]).reshape(HS, 1),
            "enct": np.ascontiguousarray(E.T.reshape(128, 16, 128)),
            "encc": np.ascontiguousarray(
                E.reshape(16, 128, 128).transpose(1, 0, 2)),
            "woh": np.ascontiguousarray(
                Wc.T[:H].reshape(KH, 128, NVT, VT).transpose(2, 1, 0, 3)
                * SCALE_W).astype(F8NP),
            "woc": np.ascontiguousarray(
                Wc.T[H:].reshape(KH, 128, NVT, VT).transpose(2, 1, 0, 3)
                * SCALE_W).astype(F8NP),
            "bo": np.ascontiguousarray(
                bp[c * VPC:(c + 1) * VPC].reshape(128, LT)),
        }
        in_maps.append(m)
    return idx, in_maps


def assemble_outputs(results):
    logits = np.concatenate(
        [results[c]["logits_out"] for c in range(NC_)])[:V].reshape(1, V)
    context = results[0]["ctx_out"].reshape(1, H).astype(np.float32)
    hidden = results[0]["hidden_out"].reshape(2, 1, H).astype(np.float32)
    attn = np.ascontiguousarray(
        results[0]["attn_out"].T).reshape(1, 1, S).astype(np.float32)
    return (logits.astype(np.float32), context, hidden, attn)


def run(inputs: dict, trace: bool = False, w_bufs: int = 8):
    idx, in_maps = prep_inputs(**inputs)
    nc = build_graph(idx, w_bufs=w_bufs)
    res = run_bass_kernel_spmd(nc, in_maps, list(range(NC_)), trace=trace)
    return assemble_outputs(res.results), res


def kernel(**inputs):
    out, _ = run(inputs, trace=False)
    return out
